# revision 24
# baseline (speedup 1.0000x reference)
"""CANModule forward kernel for 8 Trainium2 NeuronCores.

The reference computes
    new_place = relu(place_cells + ec @ W_ec + sum_i grid_i @ W_mec_i)
(the MEC grid updates are computed-then-deleted in the reference — dead
code — so W_gh*/W_gg* never need to reach the device).

Strategy: shard the HPC output dim (8192) column-wise across 8 cores
(1024 cols each).  Per core everything folds into ONE accumulated
matmul chain:
    A  = [ec (4x4096) | broadcast(concat(grids)) (4x7168)]      # [4, 11264]
    Wc = [W_ec ; W_mec0 ; W_mec1 ; W_mec2][:, shard]            # [11264, 1024]
    out_shard = relu(A @ Wc + place[shard])                     # [4, 1024]
The place bias is folded in as a K=1 matmul with a ones vector.
PE accumulates into a single [4, 1024] PSUM tile; ACT applies the relu.

Default dtype is fp16 (PE runs fp16 in one pass vs two for fp32, and the
HBM traffic halves; measured end-to-end error ~1e-4 relative). In fp16
the whole 22 MiB weight shard is SBUF-resident (11 tiles, no reuse), so
every instruction carries at most one semaphore wait. The fp32 fallback
streams weights through an 8-slot pool instead.
"""

import numpy as np

import concourse.bass as bass
import concourse.mybir as mybir
import concourse.tile as tile
from concourse.bass_utils import run_bass_kernel_spmd

N_CORES = 8
B = 4
EC = 4096
MECS = (1024, 2048, 4096)
HPC = 8192
SHARD = HPC // N_CORES          # 1024 output cols per core
K_TOTAL = EC + sum(MECS)        # 11264 contraction rows
P = 128
KC = K_TOTAL // P               # 88 K-chunks
NSPLIT = 512                    # matmul free dim = one fp32 PSUM bank

# layout of the packed per-core constants tensor "cst" [128, CST_F]:
#   cols [0, KC*B)              : swizzled A.T  (lhsT slices, [128, 4] per K-chunk)
#   cols [KC*B, KC*B + B)       : row 0 holds B ones (lhsT of the bias matmul)
#   cols [KC*B + B, CST_F)      : row 0 holds place_cells shard [1, SHARD]
ONES_OFF = KC * B
PL_OFF = KC * B + B
CST_F = KC * B + B + SHARD

CONFIG = {"trace": False, "dtype": "f16", "strip_ceremony": False}
_CACHE = {}


def _dts():
    if CONFIG["dtype"] == "f16":
        return mybir.dt.float16, np.float16
    return mybir.dt.float32, np.float32


def _build():
    DT, _ = _dts()
    resident = CONFIG["dtype"] == "f16"
    # fp16: whole W shard lives in SBUF (22 x 8KB/partition tiles). Groups of
    # 4 K-chunks (1 MiB DMAs) keep PE idle gaps under the ~3.4us HAM
    # re-throttle window so the PE stays at 2.4 GHz.
    # fp32: stream through an 8-slot pool (one slot per DMASW sem lane so the
    # WAW dep is same-lane).
    grp = 4
    ngrp = KC // grp
    bufs = ngrp if resident else 8

    nc = bass.Bass()
    cst = nc.dram_tensor("cst", [P, CST_F], DT, kind="ExternalInput")
    w = nc.dram_tensor("w", [K_TOTAL, SHARD], DT, kind="ExternalInput")
    out = nc.dram_tensor("out", [B, SHARD], mybir.dt.float32, kind="ExternalOutput")

    with tile.TileContext(nc) as tc:
        with (
            tc.tile_pool(name="const", bufs=1) as const_pool,
            tc.tile_pool(name="wload", bufs=bufs) as w_pool,
            tc.tile_pool(name="outp", bufs=1) as o_pool,
            tc.tile_pool(name="acc", bufs=1, space="PSUM") as ps_pool,
        ):
            cst_t = const_pool.tile([P, CST_F], DT)
            ps = ps_pool.tile([B, SHARD], mybir.dt.float32)

            # cst rides at the head of the ACT HWDGE ring so it does not
            # delay W group 0 at the head of the SP ring — the first W
            # matmul gates on max(cst, group 0).
            nc.scalar.dma_start(cst_t[:], cst[:])

            # place bias first (as a K=1 rank-1 update ones[4].T @ place_shard)
            # so the first matmul depends on only the cst DMA; later matmuls
            # then carry at most one sem wait each (their W-group DMA) —
            # Matmult supports a single sync-wait command.
            for j in range(SHARD // NSPLIT):
                nc.tensor.matmul(
                    ps[:, NSPLIT * j : NSPLIT * (j + 1)],
                    cst_t[0:1, ONES_OFF : ONES_OFF + B],
                    cst_t[0:1, PL_OFF + NSPLIT * j : PL_OFF + NSPLIT * (j + 1)],
                    start=True,
                    stop=False,
                )
            w_r = w.rearrange("(g c p) n -> g p c n", c=grp, p=P)
            for g in range(ngrp):
                wt = w_pool.tile([P, grp, SHARD], DT)
                if resident:
                    # alternate the two HWDGE rings (SP / ACT) so descriptor
                    # generation is not single-ring-limited
                    eng = nc.sync if g % 2 == 0 else nc.scalar
                    eng.dma_start(wt[:], w_r[g])
                else:
                    # SWDGE: HWDGE's direct2D pseudo-op can't carry the extra
                    # slot-reuse wait
                    nc.gpsimd.dma_start(wt[:], w_r[g])
                if resident and g == ngrp - 1:
                    # last group: run each bank's chunks contiguously (the
                    # same-bank run pipelines at ~216ns/mm vs 379 serialized)
                    # so the post-stream PE trail is short, and close bank 0
                    # first so its relu overlaps bank 1's matmuls
                    for j in range(SHARD // NSPLIT):
                        for c in range(grp):
                            k = g * grp + c
                            nc.tensor.matmul(
                                ps[:, NSPLIT * j : NSPLIT * (j + 1)],
                                cst_t[:, B * k : B * (k + 1)],
                                wt[:, c, NSPLIT * j : NSPLIT * (j + 1)],
                                start=False,
                                stop=(k == KC - 1),
                            )
                else:
                    for c in range(grp):
                        k = g * grp + c
                        lhsT = cst_t[:, B * k : B * (k + 1)]
                        for j in range(SHARD // NSPLIT):
                            nc.tensor.matmul(
                                ps[:, NSPLIT * j : NSPLIT * (j + 1)],
                                lhsT,
                                wt[:, c, NSPLIT * j : NSPLIT * (j + 1)],
                                start=False,
                                stop=(k == KC - 1),
                            )
            o_t = o_pool.tile([B, SHARD], mybir.dt.float32)
            for j in range(SHARD // NSPLIT):
                nc.scalar.activation(
                    o_t[:, NSPLIT * j : NSPLIT * (j + 1)],
                    ps[:, NSPLIT * j : NSPLIT * (j + 1)],
                    mybir.ActivationFunctionType.Relu,
                )
            nc.sync.dma_start(out[:], o_t[:])

    _strip_redundant_waits(nc)
    if CONFIG["strip_ceremony"]:
        _strip_ceremony(nc)
    return nc


def _strip_ceremony(nc):
    """Remove the all-engine butterfly barriers that bracket the kernel.

    The start barrier only aligns engine boot; every data dependency in this
    kernel is carried by absolute-valued semaphore waits from a zeroed sem
    file, so engines may enter their streams unaligned. At the tail, keep the
    quiesce drain + the semaphore range-clear (needed if the NEFF is ever
    re-executed) but drop the second butterfly after it — each engine's
    stream simply ends.
    """
    blocks = nc.m.functions[0].blocks
    b0 = blocks[0]
    drop = [
        n
        for n, i in enumerate(b0.instructions)
        if type(i).__name__ in ("InstDrain", "InstEventSemaphore")
    ]
    for n in reversed(drop):
        del b0.instructions[n]

    end = blocks[-1]
    isa_idx = [
        n for n, i in enumerate(end.instructions) if type(i).__name__ == "InstISA"
    ]
    if isa_idx:
        for n in range(len(end.instructions) - 1, isa_idx[-1], -1):
            del end.instructions[n]


def _emit_group_mms(nc, cst_t, ps, wt, g, grp):
    for c in range(grp):
        k = g * grp + c
        lhsT = cst_t[:, B * k : B * (k + 1)]
        for j in range(SHARD // NSPLIT):
            nc.tensor.matmul(
                ps[:, NSPLIT * j : NSPLIT * (j + 1)],
                lhsT,
                wt[:, c, NSPLIT * j : NSPLIT * (j + 1)],
                start=False,
                stop=(k == KC - 1),
            )


def _strip_redundant_waits(nc):
    """Work around Tile's non-transitively-minimal sem assignment: the DMA /
    Matmult / Drain pseudo-ops encode a single sync wait, but Tile can emit
    more.

    1. Slot-reusing W DMAs get {PE >= x, DMASW_k >= 16m}. The DMASW_k wait
       (previous same-slot DMA fully landed) is implied by PE >= x: the
       matmuls counted by PE >= x read that slot's old contents and were
       themselves gated on DMASW_k >= 16m; PE is in-order.
    2. The end-of-kernel quiesce drain waits on every proc lane, but the
       kernel is one dependency chain ending in the output-store DMA:
       store waits ACT, ACT waits PE>=all matmuls, each matmul waited its
       W-load DMA. "Store landed" implies everything else.
    """
    insts = [i for blk in nc.m.functions[0].blocks for i in blk.instructions]
    for inst in insts:
        ty = type(inst).__name__
        si = inst.sync_info
        if si is None or len(si.on_wait) <= 1:
            continue
        if ty == "InstDMACopy":
            own_lanes = {u.ant_name for u in si.on_update}
            waits = list(si.on_wait)
            self_lane = [w for w in waits if w.ant_name in own_lanes]
            engine = [
                w
                for w in waits
                if w not in self_lane
                and w.ant_name.split("_")[0] in ("PE", "Activation", "DVE", "Pool", "SP")
            ]
            rest = [w for w in waits if w not in engine and w not in self_lane]
            if len(engine) == 1 and self_lane and not rest:
                si.on_wait = engine
                continue
        if ty in ("InstDMACopy", "InstMatmult"):
            raise RuntimeError(
                f"{inst.name} ({ty}) still has {len(si.on_wait)} waits: {si}"
            )

    store = [i for i in insts if type(i).__name__ == "InstDMACopy"][-1]
    assert store.sync_info and len(store.sync_info.on_update) == 1
    lane = store.sync_info.on_update[0].ant_name
    cum = 0
    for i in insts:
        if i.sync_info:
            cum += sum(
                u.update_value for u in i.sync_info.on_update if u.ant_name == lane
            )
    for inst in insts:
        if type(inst).__name__ != "InstDrain":
            continue
        si = inst.sync_info
        if si is None or len(si.on_wait) <= 1:
            continue
        keep = [w for w in si.on_wait if w.ant_name == lane and w.wait_value == cum]
        assert keep, f"drain {inst.name} lacks the store-lane wait (cum={cum}): {si}"
        si.on_wait = keep[:1]


def kernel(**inputs):
    _, np_dt = _dts()
    ec = np.asarray(inputs["ec_activations"], dtype=np.float32)
    place = np.asarray(inputs["place_cells"], dtype=np.float32)
    grids = [np.asarray(inputs[f"grid{i}"], dtype=np.float32) for i in range(3)]
    W_ec = np.asarray(inputs["W_ec"], dtype=np.float32)
    W_mec = [np.asarray(inputs[f"W_mec{i}"], dtype=np.float32) for i in range(3)]

    X = np.concatenate(grids, axis=1)                                   # [1, 7168]
    A = np.concatenate([ec, np.broadcast_to(X, (B, X.shape[1]))], 1)    # [4, 11264]
    # pre-swizzle A.T into the SBUF layout [p, (k m)] so the device DMA is
    # a plain contiguous copy
    aT_sw = np.ascontiguousarray(
        A.T.reshape(KC, P, B).transpose(1, 0, 2)
    ).reshape(P, KC * B)

    W_all = np.concatenate([W_ec] + W_mec, axis=0).astype(np_dt)        # [11264, 8192]

    key = "nc_" + CONFIG["dtype"]
    nc = _CACHE.get(key)
    if nc is None:
        nc = _CACHE[key] = _build()

    in_maps = []
    for c in range(N_CORES):
        cols = slice(SHARD * c, SHARD * (c + 1))
        cst = np.zeros((P, CST_F), np_dt)
        cst[:, :ONES_OFF] = aT_sw
        cst[0, ONES_OFF:PL_OFF] = 1.0
        cst[0, PL_OFF:] = place[0, cols]
        in_maps.append({
            "cst": cst,
            "w": np.ascontiguousarray(W_all[:, cols]),
        })
    res = run_bass_kernel_spmd(
        nc, in_maps, core_ids=list(range(N_CORES)), trace=CONFIG["trace"]
    )
    _CACHE["last_results"] = res
    return np.concatenate([r["out"] for r in res.results], axis=1)
